# revision 24
# baseline (speedup 1.0000x reference)
"""PhaseFieldPredictor on 8 Trainium2 NeuronCores via Bass/Tile.

Sharding: core k -> (batch b=k//4, row-block rb=k%4). Each core computes a
72-row slab (64 owned rows + 4 halo rows each side, zero-padded off-grid) of
the 256x256 grid: per-node 2-layer LSTM (T=5), fc1, 4 gated-GNN layers, head.

Key structure on-device (per core):
 - LSTM is "gate-major": nodes processed in quads of 4 chunks x 512 nodes;
   chunk q lives on SBUF partitions 32q:32q+32, and each gate G of all 4
   chunks forms one 128-partition tile => full-width ACT/DVE ops.
   Matmuls are 32x32 diagonal tile_position=(32q,32q) ops; L0 bias rides an
   ones-row in the x tile (K=11), L1 biases ride the activation bias port.
 - GNN: the 8-neighbor gaussian-gated conv is exactly a separable
   [g1,1,g1] (x) [g1,1,g1] stencil (g1 = exp(-1/(g^2+1e-8)), diag weight
   g1^2) followed by a 64x64 channel matmul.  The W-direction pass is fused
   into the PE as 3 shifted accumulating matmuls (weights {g1*W, W, g1*W});
   the H-direction pass + bias + relu run on DVE as 3 fused ops.
   Node layout: channels on partitions, two row-bands stacked (band A rows
   0:40 on partitions 0:64, band B rows 32:72 on partitions 64:128), each row
   padded to 258 cols with zero guard columns.
"""
import os
import sys

import numpy as np

for _p in ("/opt/trn_rl_repo", "/root/.axon_site/_ro/trn_rl_repo"):
    if os.path.isdir(_p) and _p not in sys.path:
        sys.path.insert(0, _p)

import ml_dtypes

BF = ml_dtypes.bfloat16

# ---------------- configuration ----------------
GRID = 256
T, C, HH, WID, KW, OUTC = 5, 10, 32, 64, 32, 10
DEPTH = 4
NCORES = 8

# torch gate order in weight rows: i,f,g,o ; our slot order: i,f,o,g
_PERM = np.r_[0:32, 32:64, 96:128, 64:96]


def _geom(R):
    """Row geometry for a slab of R rows (R-8 owned)."""
    own = R - 8
    arows = own // 2 + 8          # band A rows 0:arows
    b0 = own // 2                 # band B rows b0:R
    brows = R - b0
    assert arows == brows
    return own, arows, b0


def _patch_drain(TileContext):
    """Walrus in this container rejects the Tile exit-drain's wide sem-wait
    list ('Too many sync wait commands').  Split the waits over chained
    sync-engine nops (<=4 waits each, strict FIFO on the sequencer), then
    emit a bare drain."""
    if getattr(TileContext, "_drain_patched", False):
        return
    from concourse.vector_clock import ScopedClock, VectorClock

    def _drain_and_barrier(self, tick_clock, wait_clock):
        gc = tick_clock.global_clock
        n = len(gc)
        for lo in range(0, n, 1):
            sub = [0] * n
            any_set = False
            for i in range(lo, min(lo + 1, n)):
                sub[i] = gc[i]
                any_set = any_set or gc[i] > 0
            if not any_set:
                continue
            nop = self.nc.sync.nop(nofuse=True)
            wait_clock.add_sem_waits(nop.ins,
                                     ScopedClock({None: VectorClock(sub)}))
        self.nc.sync.drain()
        self.nc.all_engine_barrier()
        assert self.sems is not None
        popped = self.nc._tile_sem_poison_stack.pop()
        assert popped is self._sem_poison
        self.nc.clear_and_free_semaphores(list(self.sems.allocated().values()))
        self.nc.all_engine_barrier()

    TileContext._drain_and_barrier = _drain_and_barrier
    TileContext._drain_patched = True


def _split_wide_waits(nc, mybir):
    """Walrus codegen in this container caps sem-waits per instruction (1 for
    DMA pseudo-instructions, small for others).  Move wide wait lists onto
    preceding same-engine nops (<=4 waits each; sequencers are in-order, so
    the fence is preserved)."""
    n_fix = 0
    for bb in nc.m.functions[0].blocks:
        insts = bb.instructions
        k = 0
        while k < len(insts):
            ins = insts[k]
            si = ins.sync_info
            if si is not None:
                waits = list(si.on_wait)
                if len(waits) > 1:
                    # keep the last wait on the instruction itself
                    for w in waits[:-1]:
                        nop = mybir.InstNoOp(
                            name=f"I-wfix-{n_fix}", ins=[], outs=[])
                        n_fix += 1
                        nop.engine = ins.engine
                        nop.sync_info = mybir.SyncInfo(
                            on_wait=[w], on_update=[])
                        insts.insert(k, nop)
                        k += 1
                    ins.sync_info = mybir.SyncInfo(
                        on_wait=[waits[-1]], on_update=list(si.on_update))
            k += 1
    return n_fix


def build_nc(R=72, CW=GRID, debug=False, fix_waits=True):
    import concourse.bass as bass
    import concourse.mybir as mybir

    from concourse.tile import TileContext
    _patch_drain(TileContext)

    F32 = mybir.dt.float32
    BF16 = mybir.dt.bfloat16
    AF = mybir.ActivationFunctionType
    ALU = mybir.AluOpType

    own, arows, b0 = _geom(R)
    CP = CW + 2                   # padded row pitch
    FD = arows * CP               # band free dim
    NQ = R // 8                   # quads
    nchunk2 = R // 2              # 2-row chunks in slab

    nc = bass.Bass()
    x_in = nc.declare_dram_parameter("x", [R // 8, T, 128, 512], BF16, isOutput=False)
    kbf = nc.declare_dram_parameter("kbf", [128, 1136], BF16, isOutput=False)
    kf32 = nc.declare_dram_parameter("kf32", [128, 12], F32, isOutput=False)
    out_d = nc.declare_dram_parameter("out", [own // 2, OUTC, 2, CW], F32,
                                      isOutput=True)

    with TileContext(nc) as tc:
        from contextlib import ExitStack
        with ExitStack() as es:
            cpool = es.enter_context(tc.tile_pool(name="const", bufs=1))
            bpool = es.enter_context(tc.tile_pool(name="band", bufs=1))
            wpool = es.enter_context(tc.tile_pool(name="work", bufs=3))
            xpool = es.enter_context(tc.tile_pool(name="xst", bufs=2))
            spool = es.enter_context(tc.tile_pool(name="state", bufs=2))
            gwpool = es.enter_context(tc.tile_pool(name="gwork", bufs=2))
            zps = ExitStack()
            zpool = zps.enter_context(tc.tile_pool(name="zps", bufs=1, space="PSUM"))

            # ---- constants (host-packed, two contiguous DMAs) ----
            # bf16 col map: w0x 0:128 | w0h 128:256 | w1i 256:384 |
            #   w1h 384:512 | wfc1 512:576 | wc 576:832 | wg 832:1088 |
            #   wfc2 1088:1120 | wfc3 1120:1136
            tKB = cpool.tile([128, 1136], BF16, tag="tKB")
            tKF = cpool.tile([128, 12], F32, tag="tKF")
            nc.sync.dma_start(out=tKB[:], in_=kbf[:])
            nc.sync.dma_start(out=tKF[:], in_=kf32[:])
            tW0x = tKB[:, 0:128]
            tW0h = tKB[:, 128:256]
            tW1i = tKB[:, 256:384]
            tW1h = tKB[:, 384:512]
            tWfc1 = tKB[:, 512:576]
            tWc = tKB[:, 576:832]
            tWg = tKB[:, 832:1088]
            tWfc2 = tKB[:, 1088:1120]
            tWfc3 = tKB[:, 1120:1136]
            # f32 col map: b1t 0:4 | fc1b 4 | cbk 5:9 | fc2b 9 | fc3b 10 |
            #   corr(mask) 11
            tB1 = tKF[:, 0:4]
            tFc1b = tKF[:, 4:5]
            tCb = tKF[:, 5:9]
            tFc2b = tKF[:, 9:10]
            tFc3b = tKF[:, 10:11]
            tCorr = tKF[:, 11:12]

            # ---- band tiles ----
            feats = bpool.tile([128, FD], BF16, tag="feats")
            mstage = bpool.tile([128, FD], BF16, tag="mstage")
            tmp = bpool.tile([128, FD - 2 * CP], BF16, tag="tmp")
            nc.vector.memset(feats[:], 0.0)
            nc.vector.memset(mstage[:], 0.0)
            nc.vector.memset(tmp[:], 0.0)

            # ---- LSTM + fc1, per quad of 8 rows ----
            for j in range(NQ):
                xts = []
                for t in range(T):
                    # host pre-arranged: parts 32q:32q+10 = x channels of chunk
                    # q (2 rows), part 32q+10 = ones row carrying the L0 bias
                    xt = xpool.tile([128, 512], BF16, tag=f"xt{t}")
                    nc.sync.dma_start(out=xt[:], in_=x_in[j, t])
                    xts.append(xt)

                h0 = spool.tile([128, 512], BF16, tag="h0")
                c0 = spool.tile([128, 512], BF16, tag="c0")
                h1 = spool.tile([128, 512], BF16, tag="h1")
                c1 = spool.tile([128, 512], BF16, tag="c1")
                nc.vector.memset(h0[:], 0.0)
                nc.vector.memset(c0[:], 0.0)
                nc.vector.memset(h1[:], 0.0)
                nc.vector.memset(c1[:], 0.0)

                for t in range(T):
                    # ----- layer 0 -----
                    z0 = zpool.tile([128, 2048], F32, tag="z0")
                    for G in range(4):
                        for q in range(4):
                            p = 32 * q
                            nc.tensor.matmul(
                                z0[p:p + 32, 512 * G:512 * G + 512],
                                lhsT=tW0x[p:p + 11, 32 * G:32 * G + 32],
                                rhs=xts[t][p:p + 11, :],
                                start=True, stop=False, tile_position=(p, p))
                            nc.tensor.matmul(
                                z0[p:p + 32, 512 * G:512 * G + 512],
                                lhsT=tW0h[p:p + 32, 32 * G:32 * G + 32],
                                rhs=h0[p:p + 32, :],
                                start=False, stop=True, tile_position=(p, p))
                    sig0 = wpool.tile([128, 1536], BF16, tag="sig0")
                    tg0 = wpool.tile([128, 512], BF16, tag="tg0")
                    nc.scalar.activation(sig0[:], z0[:, 0:1536], AF.Sigmoid)
                    nc.scalar.activation(tg0[:], z0[:, 1536:2048], AF.Tanh)
                    pp = wpool.tile([128, 512], BF16, tag="pp")
                    qq = wpool.tile([128, 512], BF16, tag="qq")
                    nc.vector.tensor_mul(pp[:], tg0[:], sig0[:, 0:512])
                    nc.vector.tensor_mul(qq[:], c0[:], sig0[:, 512:1024])
                    nc.vector.tensor_add(c0[:], pp[:], qq[:])
                    tc0 = wpool.tile([128, 512], BF16, tag="tc0")
                    nc.scalar.activation(tc0[:], c0[:], AF.Tanh)
                    nc.vector.tensor_mul(h0[:], sig0[:, 1024:1536], tc0[:])

                    # ----- layer 1 -----
                    z1 = zpool.tile([128, 2048], F32, tag="z1")
                    for G in range(4):
                        for q in range(4):
                            p = 32 * q
                            nc.tensor.matmul(
                                z1[p:p + 32, 512 * G:512 * G + 512],
                                lhsT=tW1i[p:p + 32, 32 * G:32 * G + 32],
                                rhs=h0[p:p + 32, :],
                                start=True, stop=False, tile_position=(p, p))
                            nc.tensor.matmul(
                                z1[p:p + 32, 512 * G:512 * G + 512],
                                lhsT=tW1h[p:p + 32, 32 * G:32 * G + 32],
                                rhs=h1[p:p + 32, :],
                                start=False, stop=True, tile_position=(p, p))
                    sig1 = wpool.tile([128, 1536], BF16, tag="sig1")
                    tg1 = wpool.tile([128, 512], BF16, tag="tg1")
                    for G in range(3):
                        nc.scalar.activation(sig1[:, 512 * G:512 * G + 512],
                                             z1[:, 512 * G:512 * G + 512],
                                             AF.Sigmoid, bias=tB1[:, G:G + 1])
                    nc.scalar.activation(tg1[:], z1[:, 1536:2048], AF.Tanh,
                                         bias=tB1[:, 3:4])
                    pp1 = wpool.tile([128, 512], BF16, tag="pp1")
                    qq1 = wpool.tile([128, 512], BF16, tag="qq1")
                    nc.vector.tensor_mul(pp1[:], tg1[:], sig1[:, 0:512])
                    nc.vector.tensor_mul(qq1[:], c1[:], sig1[:, 512:1024])
                    nc.vector.tensor_add(c1[:], pp1[:], qq1[:])
                    tc1 = wpool.tile([128, 512], BF16, tag="tc1")
                    nc.scalar.activation(tc1[:], c1[:], AF.Tanh)
                    nc.vector.tensor_mul(h1[:], sig1[:, 1024:1536], tc1[:])

                # ----- fc1 for this quad's 4 chunks -----
                fcp = zpool.tile([128, 2048], F32, tag="z0")
                for q in range(4):
                    m = 4 * j + q          # 2-row chunk index; rows 2m:2m+2
                    p = 32 * q
                    for band in (0, 1):
                        if band == 0 and 2 * m + 2 > arows:
                            continue
                        if band == 1 and 2 * m < b0:
                            continue
                        cs = 64 * band
                        nc.tensor.matmul(
                            fcp[cs:cs + 64, 512 * q:512 * q + 512],
                            lhsT=tWfc1[p:p + 32, :],
                            rhs=h1[p:p + 32, :],
                            start=True, stop=True, tile_position=(p, cs))
                        row = 2 * m - band * b0
                        dst = feats[cs:cs + 64, row * CP:(row + 2) * CP] \
                            .rearrange("p (r w) -> p r w", r=2)[:, :, 1:1 + CW]
                        nc.vector.tensor_scalar(
                            dst, fcp[cs:cs + 64, 512 * q:512 * q + 512]
                            .rearrange("p (r w) -> p r w", r=2),
                            tFc1b[cs:cs + 64, 0:1], 0.0, ALU.add, ALU.max)

            # ---- halo mask: zero off-grid rows (per-core 0/1 vector) ----
            def mask_halo():
                for band, r_lo in ((0, 0), (1, arows - 4)):
                    cs = 64 * band
                    sl = feats[cs:cs + 64, r_lo * CP:(r_lo + 4) * CP]
                    nc.vector.tensor_scalar(sl, sl, tCorr[cs:cs + 64, 0:1],
                                            None, ALU.mult)
            mask_halo()
            if debug:
                dbg = nc.declare_dram_parameter(
                    "dbg", [DEPTH + 1, 128, FD], F32, isOutput=True)
                nc.gpsimd.dma_start(out=dbg[0], in_=feats[:])

            zps.close()  # free LSTM psum banks
            gpool = es.enter_context(tc.tile_pool(name="gps", bufs=2, space="PSUM"))
            hpool = es.enter_context(tc.tile_pool(name="hps", bufs=2, space="PSUM"))

            # ---- GNN layers ----
            nck = (FD + 511) // 512
            for k in range(DEPTH):
                if k > 0:
                    mask_halo()
                wc = tWc[:, 64 * k:64 * k + 64]
                wg = tWg[:, 64 * k:64 * k + 64]
                for ci in range(nck):
                    lo = 512 * ci
                    ln = min(512, FD - lo)
                    mp = gpool.tile([128, 512], F32, tag="mp")
                    for band in (0, 1):
                        cs = 64 * band
                        tp = (cs, cs)
                        llo = max(lo - 1, 0)
                        rn = ln if lo + ln < FD else ln - 1
                        nc.tensor.matmul(
                            mp[cs:cs + 64, 0:ln],
                            lhsT=wc[cs:cs + 64, :],
                            rhs=feats[cs:cs + 64, lo:lo + ln],
                            start=True, stop=False, tile_position=tp)
                        nc.tensor.matmul(
                            mp[cs:cs + 64, llo - lo + 1:ln],
                            lhsT=wg[cs:cs + 64, :],
                            rhs=feats[cs:cs + 64, llo:lo + ln - 1],
                            start=False, stop=False, tile_position=tp)
                        nc.tensor.matmul(
                            mp[cs:cs + 64, 0:rn],
                            lhsT=wg[cs:cs + 64, :],
                            rhs=feats[cs:cs + 64, lo + 1:lo + 1 + rn],
                            start=False, stop=True, tile_position=tp)
                    nc.scalar.activation(mstage[:, lo:lo + ln],
                                         mp[:, 0:ln], AF.Copy)
                # H-pass: rows 1..arows-1 ; out = relu(m + g1*(up+dn) + b)
                nfd = FD - 2 * CP
                nc.vector.tensor_add(tmp[:], mstage[:, 0:nfd],
                                     mstage[:, 2 * CP:FD])
                nc.vector.scalar_tensor_tensor(
                    tmp[:], tmp[:], GW1[k], mstage[:, CP:FD - CP],
                    ALU.mult, ALU.add)
                if k != DEPTH - 1:
                    nc.vector.tensor_scalar(feats[:, CP:FD - CP], tmp[:],
                                            tCb[:, k:k + 1], 0.0,
                                            ALU.add, ALU.max)
                else:
                    nc.vector.tensor_scalar(feats[:, CP:FD - CP], tmp[:],
                                            tCb[:, k:k + 1], None, ALU.add)
                # re-zero guard cols (both bands, all rows)
                nc.vector.memset(
                    feats.rearrange("p (r w) -> p r w", r=arows)[:, :, 0:1], 0.0)
                nc.vector.memset(
                    feats.rearrange("p (r w) -> p r w", r=arows)[:, :, CP - 1:CP],
                    0.0)
                if debug:
                    nc.gpsimd.dma_start(out=dbg[k + 1], in_=feats[:])

            # ---- head: owned rows = band rows 4 : 4+own/2 on each band ----
            for m in range(own // 4):
                lo = (4 + 2 * m) * CP
                hp = hpool.tile([128, 512], F32, tag="hp")
                for band in (0, 1):
                    cs = 64 * band
                    rhs_ap = feats[cs:cs + 64, lo:lo + 2 * CP] \
                        .rearrange("p (r w) -> p r w", r=2)[:, :, 1:1 + CW]
                    nc.tensor.matmul(
                        hp[cs:cs + 32, 0:512],
                        lhsT=tWfc2[cs:cs + 64, :],
                        rhs=rhs_ap,
                        start=True, stop=True, tile_position=(cs, cs))
                r2 = gwpool.tile([128, 512], BF16, tag="r2")
                for band in (0, 1):
                    cs = 64 * band
                    nc.vector.tensor_scalar(r2[cs:cs + 32, :], hp[cs:cs + 32, :],
                                            tFc2b[cs:cs + 32, 0:1], 0.0,
                                            ALU.add, ALU.max)
                op3 = hpool.tile([128, 512], F32, tag="op3")
                for band in (0, 1):
                    cs = 64 * band
                    nc.tensor.matmul(
                        op3[cs:cs + OUTC, 0:512],
                        lhsT=tWfc3[cs:cs + 32, 0:OUTC],
                        rhs=r2[cs:cs + 32, :],
                        start=True, stop=True, tile_position=(cs, cs))
                ot = gwpool.tile([128, 512], F32, tag="ot")
                nc.vector.memset(ot[:], 0.0)
                for band in (0, 1):
                    cs = 64 * band
                    nc.vector.tensor_scalar(ot[cs:cs + OUTC, :],
                                            op3[cs:cs + OUTC, :],
                                            tFc3b[cs:cs + OUTC, 0:1],
                                            None, ALU.add)
                # out block: band A -> m ; band B -> own/4 + m
                for band in (0, 1):
                    cs = 64 * band
                    osrc = ot[cs:cs + OUTC, :].rearrange(
                        "p (r w) -> p r w", r=2)
                    nc.sync.dma_start(
                        out=out_d[band * (own // 4) + m], in_=osrc)

    if fix_waits:
        _split_wide_waits(nc, mybir)
    return nc


GW1 = [1.0] * DEPTH  # per-layer g1 scalars, set by host before build


def _prep_shared(Wih0, Whh0, bih0, bhh0, Wih1, Whh1, bih1, bhh1,
                 fc1_w, fc1_b, conv_w, conv_b, gparam, fc2_w, fc2_b,
                 fc3_w, fc3_b):
    """Weight/bias tiles shared by all cores; returns (dict, c_feats, g1)."""
    Wih0p, Whh0p = Wih0[_PERM], Whh0[_PERM]
    Wih1p, Whh1p = Wih1[_PERM], Whh1[_PERM]
    b0p = (bih0 + bhh0)[_PERM]
    b1p = (bih1 + bhh1)[_PERM]

    w0x = np.zeros((128, 128), np.float32)
    w0h = np.zeros((128, 128), np.float32)
    w1i = np.zeros((128, 128), np.float32)
    w1h = np.zeros((128, 128), np.float32)
    b1t = np.zeros((128, 4), np.float32)
    for q in range(4):
        for G in range(4):
            w0x[32 * q:32 * q + 10, 32 * G:32 * G + 32] = \
                Wih0p[32 * G:32 * G + 32].T
            w0x[32 * q + 10, 32 * G:32 * G + 32] = b0p[32 * G:32 * G + 32]
            w0h[32 * q:32 * q + 32, 32 * G:32 * G + 32] = \
                Whh0p[32 * G:32 * G + 32].T
            w1i[32 * q:32 * q + 32, 32 * G:32 * G + 32] = \
                Wih1p[32 * G:32 * G + 32].T
            w1h[32 * q:32 * q + 32, 32 * G:32 * G + 32] = \
                Whh1p[32 * G:32 * G + 32].T
            b1t[32 * q:32 * q + 32, G] = b1p[32 * G:32 * G + 32]

    wfc1 = np.zeros((128, 64), np.float32)
    for q in range(4):
        wfc1[32 * q:32 * q + 32] = fc1_w.T
    fc1bt = np.tile(fc1_b, 2)[:, None].astype(np.float32)

    g1 = np.exp(-1.0 / (gparam.astype(np.float64) ** 2 + 1e-8)).astype(np.float32)
    wck = np.zeros((DEPTH, 128, 64), np.float32)
    wgk = np.zeros((DEPTH, 128, 64), np.float32)
    cbk = np.zeros((DEPTH, 128, 1), np.float32)
    for k in range(DEPTH):
        wck[k, 0:64] = conv_w[k]
        wck[k, 64:128] = conv_w[k]
        wgk[k] = wck[k] * g1[k]
        cbk[k, 0:64, 0] = conv_b[k]
        cbk[k, 64:128, 0] = conv_b[k]

    wfc2 = np.zeros((128, 32), np.float32)
    wfc2[0:64] = fc2_w.T
    wfc2[64:128] = fc2_w.T
    fc2bt = np.zeros((128, 1), np.float32)
    fc2bt[0:32, 0] = fc2_b
    fc2bt[64:96, 0] = fc2_b
    wfc3 = np.zeros((128, 16), np.float32)
    wfc3[0:32, 0:OUTC] = fc3_w.T
    wfc3[64:96, 0:OUTC] = fc3_w.T
    fc3bt = np.zeros((128, 1), np.float32)
    fc3bt[0:OUTC, 0] = fc3_b
    fc3bt[64:64 + OUTC, 0] = fc3_b

    # LSTM(0-input) fixed point -> halo feats constant
    def sig(v):
        return 1.0 / (1.0 + np.exp(-v))
    h0 = c0 = h1 = c1 = np.zeros(HH, np.float32)
    for _ in range(T):
        z = h0 @ Whh0.T + bih0 + bhh0
        i_, f_, g_, o_ = np.split(z, 4)
        c0 = sig(f_) * c0 + sig(i_) * np.tanh(g_)
        h0 = sig(o_) * np.tanh(c0)
        z = h0 @ Wih1.T + bih1 + h1 @ Whh1.T + bhh1
        i_, f_, g_, o_ = np.split(z, 4)
        c1 = sig(f_) * c1 + sig(i_) * np.tanh(g_)
        h1 = sig(o_) * np.tanh(c1)
    c_feats = np.maximum(h1 @ fc1_w.T + fc1_b, 0.0).astype(np.float32)

    kbf = np.zeros((128, 1136), np.float32)
    kbf[:, 0:128] = w0x
    kbf[:, 128:256] = w0h
    kbf[:, 256:384] = w1i
    kbf[:, 384:512] = w1h
    kbf[:, 512:576] = wfc1
    kbf[:, 576:832] = wck.transpose(1, 0, 2).reshape(128, 256)
    kbf[:, 832:1088] = wgk.transpose(1, 0, 2).reshape(128, 256)
    kbf[:, 1088:1120] = wfc2
    kbf[:, 1120:1136] = wfc3
    kf32 = np.zeros((128, 12), np.float32)
    kf32[:, 0:4] = b1t
    kf32[:, 4:5] = fc1bt
    kf32[:, 5:9] = cbk.transpose(1, 0, 2).reshape(128, 4)
    kf32[:, 9:10] = fc2bt
    kf32[:, 10:11] = fc3bt
    shared = dict(kbf=kbf.astype(BF), kf32=kf32)
    return shared, c_feats, g1


def _arrange_x(xb, r0, R):
    """xb: (T, C, GRID, CW) one batch -> (R//8, T, 128, 512) quad-tile layout.
    Slab rows r0:r0+R (clamped, zero-padded); part 32q+10 = 1.0 (bias row)."""
    T_, C_, G_, CW_ = xb.shape
    slab = np.zeros((T_, C_, R, CW_), np.float32)
    lo, hi = max(r0, 0), min(r0 + R, G_)
    slab[:, :, lo - r0:hi - r0, :] = xb[:, :, lo:hi, :]
    out = np.zeros((R // 8, T_, 128, 512), np.float32)
    # (T, C, R, CW) -> quads j, chunks q (2 rows each)
    s = slab.reshape(T_, C_, R // 8, 4, 2 * CW_)
    out.reshape(R // 8, T_, 4, 32, 512)[:, :, :, 0:C_, :] = \
        s.transpose(2, 0, 3, 1, 4)
    out.reshape(R // 8, T_, 4, 32, 512)[:, :, :, C_, :] = 1.0
    return out.astype(BF)


def _host_prep(x, shared, c_feats, R=72):
    """Per-core input dicts: x slabs + halo-correction vectors."""
    own = R - 8
    B = x.shape[0]
    in_maps = []
    nblk = NCORES // B
    for core in range(NCORES):
        b, rb = core // nblk, core % nblk
        r0 = rb * own - 4
        xs = _arrange_x(x[b], r0, R)
        kf = shared["kf32"].copy()
        kf[:, 11] = 1.0
        if rb == 0:
            kf[0:64, 11] = 0.0
        if rb == nblk - 1:
            kf[64:128, 11] = 0.0
        m = dict(shared)
        m["x"] = xs
        m["kf32"] = kf
        in_maps.append(m)
    return in_maps


_CACHE = {}
TRACE = False
LAST_EXEC_NS = None
LAST_TRACE = None


def _run_in_subprocess(in_maps):
    """Run the SPMD program in a child process with a clean jax env.

    The grading/reference process often pins JAX_PLATFORMS=cpu, which breaks
    the axon PJRT compile hook; a child with a scrubbed env always sees the
    8 NeuronCores."""
    import pickle
    import subprocess
    import tempfile

    workdir = tempfile.mkdtemp(prefix="pfk_")
    inp = os.path.join(workdir, "in.pkl")
    outp = os.path.join(workdir, "out.pkl")
    with open(inp, "wb") as f:
        pickle.dump({"in_maps": in_maps, "gw1": GW1, "trace": TRACE}, f,
                    protocol=4)
    env = dict(os.environ)
    env.pop("JAX_PLATFORMS", None)
    env.pop("JAX_PLATFORM_NAME", None)
    subprocess.run([sys.executable, os.path.abspath(__file__),
                    "--worker", inp, outp], check=True, env=env)
    with open(outp, "rb") as f:
        return pickle.load(f)


def _worker(inp, outp):
    import pickle
    import time as _time
    import types

    # the trimmed axon container lacks antenv.axon_hooks (NTFF profiling);
    # stub it so trace=True degrades to no-trace instead of crashing.
    if "antenv.axon_hooks" not in sys.modules:
        stub = types.ModuleType("antenv.axon_hooks")
        stub.get_axon_ntff_profile_hook = lambda: None
        sys.modules["antenv.axon_hooks"] = stub

    with open(inp, "rb") as f:
        payload = pickle.load(f)
    global GW1, TRACE
    GW1 = payload["gw1"]
    TRACE = payload["trace"]
    from concourse.bass_utils import run_bass_kernel_spmd
    nc = build_nc(R=72)
    res = run_bass_kernel_spmd(nc, payload["in_maps"], list(range(NCORES)))
    times = []
    n_rep = int(os.environ.get("KREPS", "3"))
    for _ in range(n_rep):
        t0 = _time.perf_counter()
        res = run_bass_kernel_spmd(nc, payload["in_maps"], list(range(NCORES)))
        times.append(_time.perf_counter() - t0)
    out = {
        "outs": [np.asarray(res.results[i]["out"]) for i in range(NCORES)],
        "exec_time_ns": int(min(times) * 1e9) if times else None,
        "trace": None,
        "times": times,
    }
    with open(outp, "wb") as f:
        pickle.dump(out, f, protocol=4)


def kernel(x, edge_src, edge_tgt, edge_attr, Wih0, Whh0, bih0, bhh0,
           Wih1, Whh1, bih1, bhh1, fc1_w, fc1_b, conv_w, conv_b, gparam,
           fc2_w, fc2_b, fc3_w, fc3_b):
    x = np.ascontiguousarray(np.asarray(x, np.float32))
    B = x.shape[0]
    R, own = 72, 64
    shared, c_feats, g1 = _prep_shared(
        *[np.asarray(a, np.float32) for a in
          (Wih0, Whh0, bih0, bhh0, Wih1, Whh1, bih1, bhh1, fc1_w, fc1_b,
           conv_w, conv_b, gparam, fc2_w, fc2_b, fc3_w, fc3_b)])
    in_maps = _host_prep(x, shared, c_feats, R=R)

    global GW1
    GW1 = [float(v) for v in g1]

    global LAST_EXEC_NS, LAST_TRACE
    res = _run_in_subprocess(in_maps)
    LAST_EXEC_NS = res.get("exec_time_ns")
    LAST_TRACE = res.get("trace")
    nblk = NCORES // B
    full = np.zeros((B, OUTC, GRID, GRID), np.float32)
    for core in range(NCORES):
        b, rb = core // nblk, core % nblk
        o = res["outs"][core]  # (own//2, OUTC, 2, CW)
        full[b, :, rb * own:(rb + 1) * own, :] = \
            o.transpose(1, 0, 2, 3).reshape(OUTC, own, GRID)
    return full[:, None].astype(np.float32)


if __name__ == "__main__":
    if len(sys.argv) == 4 and sys.argv[1] == "--worker":
        _worker(sys.argv[2], sys.argv[3])


# revision 31
# speedup vs baseline: 1.1853x; 1.1853x over previous
"""PhaseFieldPredictor on 8 Trainium2 NeuronCores via Bass/Tile.

Sharding: core k -> (batch b=k//4, row-block rb=k%4). Each core computes a
72-row slab (64 owned rows + 4 halo rows each side, zero-padded off-grid) of
the 256x256 grid: per-node 2-layer LSTM (T=5), fc1, 4 gated-GNN layers, head.

Key structure on-device (per core):
 - LSTM is "gate-major": nodes processed in quads of 4 chunks x 512 nodes;
   chunk q lives on SBUF partitions 32q:32q+32, and each gate G of all 4
   chunks forms one 128-partition tile => full-width ACT/DVE ops.
   Matmuls are 32x32 diagonal tile_position=(32q,32q) ops; L0 bias rides an
   ones-row in the x tile (K=11), L1 biases ride the activation bias port.
 - GNN: the 8-neighbor gaussian-gated conv is exactly a separable
   [g1,1,g1] (x) [g1,1,g1] stencil (g1 = exp(-1/(g^2+1e-8)), diag weight
   g1^2) followed by a 64x64 channel matmul.  The W-direction pass is fused
   into the PE as 3 shifted accumulating matmuls (weights {g1*W, W, g1*W});
   the H-direction pass + bias + relu run on DVE as 3 fused ops.
   Node layout: channels on partitions, two row-bands stacked (band A rows
   0:40 on partitions 0:64, band B rows 32:72 on partitions 64:128), each row
   padded to 258 cols with zero guard columns.
"""
import os
import sys

import numpy as np

for _p in ("/opt/trn_rl_repo", "/root/.axon_site/_ro/trn_rl_repo"):
    if os.path.isdir(_p) and _p not in sys.path:
        sys.path.insert(0, _p)

import ml_dtypes

BF = ml_dtypes.bfloat16

# ---------------- configuration ----------------
GRID = 256
T, C, HH, WID, KW, OUTC = 5, 10, 32, 64, 32, 10
DEPTH = 4
NCORES = 8

# torch gate order in weight rows: i,f,g,o ; our slot order: i,f,o,g
_PERM = np.r_[0:32, 32:64, 96:128, 64:96]


def _geom(R):
    """Row geometry for a slab of R rows (R-8 owned)."""
    own = R - 8
    arows = own // 2 + 8          # band A rows 0:arows
    b0 = own // 2                 # band B rows b0:R
    brows = R - b0
    assert arows == brows
    return own, arows, b0


def _patch_drain(TileContext):
    """Walrus in this container rejects the Tile exit-drain's wide sem-wait
    list ('Too many sync wait commands').  Split the waits over chained
    sync-engine nops (<=4 waits each, strict FIFO on the sequencer), then
    emit a bare drain."""
    if getattr(TileContext, "_drain_patched", False):
        return
    from concourse.vector_clock import ScopedClock, VectorClock

    def _drain_and_barrier(self, tick_clock, wait_clock):
        gc = tick_clock.global_clock
        n = len(gc)
        for lo in range(0, n, 1):
            sub = [0] * n
            any_set = False
            for i in range(lo, min(lo + 1, n)):
                sub[i] = gc[i]
                any_set = any_set or gc[i] > 0
            if not any_set:
                continue
            nop = self.nc.sync.nop(nofuse=True)
            wait_clock.add_sem_waits(nop.ins,
                                     ScopedClock({None: VectorClock(sub)}))
        self.nc.sync.drain()
        self.nc.all_engine_barrier()
        assert self.sems is not None
        popped = self.nc._tile_sem_poison_stack.pop()
        assert popped is self._sem_poison
        self.nc.clear_and_free_semaphores(list(self.sems.allocated().values()))
        self.nc.all_engine_barrier()

    TileContext._drain_and_barrier = _drain_and_barrier
    TileContext._drain_patched = True


def _split_wide_waits(nc, mybir):
    """Walrus codegen in this container caps sem-waits per instruction (1 for
    DMA pseudo-instructions, small for others).  Move wide wait lists onto
    preceding same-engine nops (<=4 waits each; sequencers are in-order, so
    the fence is preserved)."""
    n_fix = 0
    for bb in nc.m.functions[0].blocks:
        insts = bb.instructions
        k = 0
        while k < len(insts):
            ins = insts[k]
            si = ins.sync_info
            if si is not None:
                waits = list(si.on_wait)
                if len(waits) > 1:
                    # keep the last wait on the instruction itself
                    for w in waits[:-1]:
                        nop = mybir.InstNoOp(
                            name=f"I-wfix-{n_fix}", ins=[], outs=[])
                        n_fix += 1
                        nop.engine = ins.engine
                        nop.sync_info = mybir.SyncInfo(
                            on_wait=[w], on_update=[])
                        insts.insert(k, nop)
                        k += 1
                    ins.sync_info = mybir.SyncInfo(
                        on_wait=[waits[-1]], on_update=list(si.on_update))
            k += 1
    return n_fix


def build_nc(R=72, CW=GRID, debug=False, fix_waits=True):
    import concourse.bass as bass
    import concourse.mybir as mybir

    from concourse.tile import TileContext
    _patch_drain(TileContext)

    F32 = mybir.dt.float32
    BF16 = mybir.dt.bfloat16
    AF = mybir.ActivationFunctionType
    ALU = mybir.AluOpType

    own, arows, b0 = _geom(R)
    CP = CW + 2                   # padded row pitch
    FD = arows * CP               # band free dim
    NQ = R // 8                   # quads
    nchunk2 = R // 2              # 2-row chunks in slab

    nc = bass.Bass()
    x_in = nc.declare_dram_parameter("x", [R // 8, T, 128, 512], BF16, isOutput=False)
    kbf = nc.declare_dram_parameter("kbf", [128, 3744], BF16, isOutput=False)
    kf32 = nc.declare_dram_parameter("kf32", [128, 12], F32, isOutput=False)
    out_d = nc.declare_dram_parameter("out", [own // 2, OUTC, 2, CW], F32,
                                      isOutput=True)

    with TileContext(nc) as tc:
        from contextlib import ExitStack
        with ExitStack() as es:
            cpool = es.enter_context(tc.tile_pool(name="const", bufs=1))
            bpool = es.enter_context(tc.tile_pool(name="band", bufs=1))
            wpool = es.enter_context(tc.tile_pool(name="work", bufs=5))
            xpool = es.enter_context(tc.tile_pool(name="xst", bufs=3))
            spool = es.enter_context(tc.tile_pool(name="state", bufs=3))
            gwpool = es.enter_context(tc.tile_pool(name="gwork", bufs=2))
            zps = ExitStack()
            zpool = zps.enter_context(tc.tile_pool(name="zps", bufs=1, space="PSUM"))

            # ---- constants (host-packed block-diagonal, two DMAs) ----
            # bf16 col map: w0x 0:512 | w0h 512:1024 | w1i 1024:1536 |
            #   w1h 1536:2048 | wfc1 2048:2112 | wc 2112:2624 | wg 2624:3136 |
            #   wfc2 3136:3200 | wfc3 3200:3232
            tKB = cpool.tile([128, 3744], BF16, tag="tKB")
            tKF = cpool.tile([128, 12], F32, tag="tKF")
            nc.sync.dma_start(out=tKB[:], in_=kbf[:])
            nc.sync.dma_start(out=tKF[:], in_=kf32[:])
            tW0x = tKB[:, 0:512]
            tW0h = tKB[:, 512:1024]
            tW1i = tKB[:, 1024:1536]
            tW1h = tKB[:, 1536:2048]
            tWfc1 = tKB[:, 2048:2112]
            tWc = tKB[:, 2112:2624]
            tWg = tKB[:, 2624:3136]
            tWfc2 = tKB[:, 3136:3200]
            tWfc3 = tKB[:, 3200:3232]
            tB1m = tKB[:, 3232:3744]
            # f32 col map: b1t 0:4 | fc1b 4 | cbk 5:9 | fc2b 9 | fc3b 10 |
            #   corr(mask) 11
            tB1 = tKF[:, 0:4]
            tFc1b = tKF[:, 4:5]
            tCb = tKF[:, 5:9]
            tFc2b = tKF[:, 9:10]
            tFc3b = tKF[:, 10:11]
            tCorr = tKF[:, 11:12]

            # ---- band tiles ----
            feats = bpool.tile([128, FD], BF16, tag="feats")
            mstage = bpool.tile([128, FD], BF16, tag="mstage")
            tmp = bpool.tile([128, FD - 2 * CP], BF16, tag="tmp")
            nc.vector.memset(feats[:], 0.0)
            nc.vector.memset(mstage[:], 0.0)
            nc.vector.memset(tmp[:], 0.0)

            # ---- LSTM + fc1, per quad of 8 rows ----
            for j in range(NQ):
                xts = []
                for t in range(T):
                    # host pre-arranged: parts 32q:32q+10 = x channels of chunk
                    # q (2 rows), part 32q+10 = ones row carrying the L0 bias
                    xt = xpool.tile([128, 512], BF16, tag=f"xt{t}")
                    nc.sync.dma_start(out=xt[:], in_=x_in[j, t])
                    xts.append(xt)

                # ----- pass A: layer 0, all T steps (h0 kept per step) ----
                h0a = spool.tile([128, 512 * T], BF16, tag="h0a")
                c0 = spool.tile([128, 512], BF16, tag="c0")
                for t in range(T):
                    z0 = zpool.tile([128, 2048], F32, tag="z0")
                    for G in (3, 0, 1, 2):
                        nc.tensor.matmul(
                            z0[:, 512 * G:512 * G + 512],
                            lhsT=tW0x[:, 128 * G:128 * G + 128],
                            rhs=xts[t][:],
                            start=True, stop=(t == 0))
                        if t > 0:
                            nc.tensor.matmul(
                                z0[:, 512 * G:512 * G + 512],
                                lhsT=tW0h[:, 128 * G:128 * G + 128],
                                rhs=h0a[:, 512 * (t - 1):512 * t],
                                start=False, stop=True)
                    sig0 = wpool.tile([128, 1536], BF16, tag="sig0")
                    tg0 = wpool.tile([128, 512], BF16, tag="tg0")
                    nc.scalar.activation(tg0[:], z0[:, 1536:2048], AF.Tanh)
                    nc.scalar.activation(sig0[:, 0:512], z0[:, 0:512],
                                         AF.Sigmoid)
                    nc.scalar.activation(sig0[:, 512:1536], z0[:, 512:1536],
                                         AF.Sigmoid)
                    if t == 0:
                        nc.vector.tensor_mul(c0[:], tg0[:], sig0[:, 0:512])
                    else:
                        pp = wpool.tile([128, 512], BF16, tag="pp")
                        qq = wpool.tile([128, 512], BF16, tag="qq")
                        nc.vector.tensor_mul(pp[:], tg0[:], sig0[:, 0:512])
                        nc.vector.tensor_mul(qq[:], c0[:], sig0[:, 512:1024])
                        nc.vector.tensor_add(c0[:], pp[:], qq[:])
                    tc0 = wpool.tile([128, 512], BF16, tag="tc0")
                    nc.scalar.activation(tc0[:], c0[:], AF.Tanh)
                    nc.vector.tensor_mul(h0a[:, 512 * t:512 * t + 512],
                                         sig0[:, 1024:1536], tc0[:])

                # ----- pass B: layer 1, all T steps ----
                h1 = spool.tile([128, 512], BF16, tag="h1")
                c1 = spool.tile([128, 512], BF16, tag="c1")
                for t in range(T):
                    z1 = zpool.tile([128, 2048], F32, tag="z1")
                    for G in (3, 0, 1, 2):
                        nc.tensor.matmul(
                            z1[:, 512 * G:512 * G + 512],
                            lhsT=tB1m[:, 128 * G:128 * G + 128],
                            rhs=xts[t][:],
                            start=True, stop=False)
                        nc.tensor.matmul(
                            z1[:, 512 * G:512 * G + 512],
                            lhsT=tW1i[:, 128 * G:128 * G + 128],
                            rhs=h0a[:, 512 * t:512 * t + 512],
                            start=False, stop=(t == 0))
                        if t > 0:
                            nc.tensor.matmul(
                                z1[:, 512 * G:512 * G + 512],
                                lhsT=tW1h[:, 128 * G:128 * G + 128],
                                rhs=h1[:],
                                start=False, stop=True)
                    sig1 = wpool.tile([128, 1536], BF16, tag="sig1")
                    tg1 = wpool.tile([128, 512], BF16, tag="tg1")
                    nc.scalar.activation(tg1[:], z1[:, 1536:2048], AF.Tanh)
                    nc.scalar.activation(sig1[:, 0:512], z1[:, 0:512],
                                         AF.Sigmoid)
                    nc.scalar.activation(sig1[:, 512:1536], z1[:, 512:1536],
                                         AF.Sigmoid)
                    if t == 0:
                        nc.vector.tensor_mul(c1[:], tg1[:], sig1[:, 0:512])
                    else:
                        pp1 = wpool.tile([128, 512], BF16, tag="pp1")
                        qq1 = wpool.tile([128, 512], BF16, tag="qq1")
                        nc.vector.tensor_mul(pp1[:], tg1[:], sig1[:, 0:512])
                        nc.vector.tensor_mul(qq1[:], c1[:], sig1[:, 512:1024])
                        nc.vector.tensor_add(c1[:], pp1[:], qq1[:])
                    tc1 = wpool.tile([128, 512], BF16, tag="tc1")
                    nc.scalar.activation(tc1[:], c1[:], AF.Tanh)
                    nc.vector.tensor_mul(h1[:], sig1[:, 1024:1536], tc1[:])

                # ----- fc1 for this quad's 4 chunks -----
                fcp = zpool.tile([128, 2048], F32, tag="z1")
                for q in range(4):
                    m = 4 * j + q          # 2-row chunk index; rows 2m:2m+2
                    p = 32 * q
                    for band in (0, 1):
                        if band == 0 and 2 * m + 2 > arows:
                            continue
                        if band == 1 and 2 * m < b0:
                            continue
                        cs = 64 * band
                        nc.tensor.matmul(
                            fcp[cs:cs + 64, 512 * q:512 * q + 512],
                            lhsT=tWfc1[p:p + 32, :],
                            rhs=h1[p:p + 32, :],
                            start=True, stop=True, tile_position=(p, cs))
                        row = 2 * m - band * b0
                        dst = feats[cs:cs + 64, row * CP:(row + 2) * CP] \
                            .rearrange("p (r w) -> p r w", r=2)[:, :, 1:1 + CW]
                        nc.vector.tensor_scalar(
                            dst, fcp[cs:cs + 64, 512 * q:512 * q + 512]
                            .rearrange("p (r w) -> p r w", r=2),
                            tFc1b[cs:cs + 64, 0:1], 0.0, ALU.add, ALU.max)

            # ---- halo mask: zero off-grid rows (per-core 0/1 vector) ----
            def mask_halo():
                for band, r_lo in ((0, 0), (1, arows - 4)):
                    cs = 64 * band
                    sl = feats[cs:cs + 64, r_lo * CP:(r_lo + 4) * CP]
                    nc.vector.tensor_scalar(sl, sl, tCorr[cs:cs + 64, 0:1],
                                            None, ALU.mult)
            mask_halo()
            if debug:
                dbg = nc.declare_dram_parameter(
                    "dbg", [DEPTH + 1, 128, FD], F32, isOutput=True)
                nc.gpsimd.dma_start(out=dbg[0], in_=feats[:])

            zps.close()  # free LSTM psum banks
            gpool = es.enter_context(tc.tile_pool(name="gps", bufs=2, space="PSUM"))
            hpool = es.enter_context(tc.tile_pool(name="hps", bufs=2, space="PSUM"))

            # ---- GNN layers ----
            nck = (FD + 511) // 512
            for k in range(DEPTH):
                if k > 0:
                    mask_halo()
                wc = tWc[:, 128 * k:128 * k + 128]
                wg = tWg[:, 128 * k:128 * k + 128]
                for ci in range(nck):
                    lo = 512 * ci
                    ln = min(512, FD - lo)
                    mp = gpool.tile([128, 512], F32, tag="mp")
                    llo = max(lo - 1, 0)
                    rn = ln if lo + ln < FD else ln - 1
                    nc.tensor.matmul(
                        mp[:, 0:ln], lhsT=wc,
                        rhs=feats[:, lo:lo + ln],
                        start=True, stop=False)
                    nc.tensor.matmul(
                        mp[:, llo - lo + 1:ln], lhsT=wg,
                        rhs=feats[:, llo:lo + ln - 1],
                        start=False, stop=False)
                    nc.tensor.matmul(
                        mp[:, 0:rn], lhsT=wg,
                        rhs=feats[:, lo + 1:lo + 1 + rn],
                        start=False, stop=True)
                    nc.scalar.activation(mstage[:, lo:lo + ln],
                                         mp[:, 0:ln], AF.Copy)
                # H-pass: rows 1..arows-1 ; out = relu(m + g1*(up+dn) + b)
                # done in column blocks so the next layer's W-pass chunks can
                # start before the whole band is swept
                nfd = FD - 2 * CP
                nblk = 4
                bsz = (nfd + nblk - 1) // nblk
                for bi in range(nblk):
                    blo = bi * bsz
                    bn = min(bsz, nfd - blo)
                    tb = tmp[:, blo:blo + bn]
                    nc.vector.tensor_add(tb, mstage[:, blo:blo + bn],
                                         mstage[:, blo + 2 * CP:blo + 2 * CP + bn])
                    nc.vector.scalar_tensor_tensor(
                        tb, tb, GW1[k], mstage[:, blo + CP:blo + CP + bn],
                        ALU.mult, ALU.add)
                    if k != DEPTH - 1:
                        nc.vector.tensor_scalar(
                            feats[:, blo + CP:blo + CP + bn], tb,
                            tCb[:, k:k + 1], 0.0, ALU.add, ALU.max)
                    else:
                        nc.vector.tensor_scalar(
                            feats[:, blo + CP:blo + CP + bn], tb,
                            tCb[:, k:k + 1], None, ALU.add)
                # re-zero guard cols (both bands, all rows)
                nc.vector.memset(
                    feats.rearrange("p (r w) -> p r w", r=arows)[:, :, 0:1], 0.0)
                nc.vector.memset(
                    feats.rearrange("p (r w) -> p r w", r=arows)[:, :, CP - 1:CP],
                    0.0)
                if debug:
                    nc.gpsimd.dma_start(out=dbg[k + 1], in_=feats[:])

            # ---- head: owned rows = band rows 4 : 4+own/2 on each band ----
            for m in range(own // 4):
                lo = (4 + 2 * m) * CP
                hp = hpool.tile([64, 512], F32, tag="hp")
                rhs_ap = feats[:, lo:lo + 2 * CP] \
                    .rearrange("p (r w) -> p r w", r=2)[:, :, 1:1 + CW]
                nc.tensor.matmul(hp[:], lhsT=tWfc2, rhs=rhs_ap,
                                 start=True, stop=True)
                r2 = gwpool.tile([64, 512], BF16, tag="r2")
                nc.vector.tensor_scalar(r2[:], hp[:], tFc2b[0:64, 0:1], 0.0,
                                        ALU.add, ALU.max)
                op3 = hpool.tile([32, 512], F32, tag="op3")
                nc.tensor.matmul(op3[:], lhsT=tWfc3[0:64, :], rhs=r2[:],
                                 start=True, stop=True)
                ot = gwpool.tile([32, 512], F32, tag="ot")
                nc.vector.tensor_scalar(ot[:], op3[:], tFc3b[0:32, 0:1],
                                        None, ALU.add)
                # out block: band A (parts 0:10) -> m ; band B (16:26) -> +own/4
                for band in (0, 1):
                    cs = 16 * band
                    osrc = ot[cs:cs + OUTC, :].rearrange(
                        "p (r w) -> p r w", r=2)
                    nc.sync.dma_start(
                        out=out_d[band * (own // 4) + m], in_=osrc)

    if fix_waits:
        _split_wide_waits(nc, mybir)
    return nc


GW1 = [1.0] * DEPTH  # per-layer g1 scalars, set by host before build


def _prep_shared(Wih0, Whh0, bih0, bhh0, Wih1, Whh1, bih1, bhh1,
                 fc1_w, fc1_b, conv_w, conv_b, gparam, fc2_w, fc2_b,
                 fc3_w, fc3_b):
    """Weight/bias tiles shared by all cores; returns (dict, c_feats, g1)."""
    Wih0p, Whh0p = Wih0[_PERM], Whh0[_PERM]
    Wih1p, Whh1p = Wih1[_PERM], Whh1[_PERM]
    b0p = (bih0 + bhh0)[_PERM]
    b1p = (bih1 + bhh1)[_PERM]

    # block-diagonal over the 4 chunk-slots: gate-G matmul lhsT is
    # (128 K x 128 M) with chunk q's block at [32q:32q+*, 128G+32q:+32]
    w0x = np.zeros((128, 512), np.float32)
    w0h = np.zeros((128, 512), np.float32)
    w1i = np.zeros((128, 512), np.float32)
    w1h = np.zeros((128, 512), np.float32)
    b1t = np.zeros((128, 4), np.float32)
    for q in range(4):
        for G in range(4):
            c0 = 128 * G + 32 * q
            w0x[32 * q:32 * q + 10, c0:c0 + 32] = Wih0p[32 * G:32 * G + 32].T
            w0x[32 * q + 10, c0:c0 + 32] = b0p[32 * G:32 * G + 32]
            w0h[32 * q:32 * q + 32, c0:c0 + 32] = Whh0p[32 * G:32 * G + 32].T
            w1i[32 * q:32 * q + 32, c0:c0 + 32] = Wih1p[32 * G:32 * G + 32].T
            w1h[32 * q:32 * q + 32, c0:c0 + 32] = Whh1p[32 * G:32 * G + 32].T
            b1t[32 * q:32 * q + 32, G] = b1p[32 * G:32 * G + 32]

    wfc1 = np.zeros((128, 64), np.float32)
    for q in range(4):
        wfc1[32 * q:32 * q + 32] = fc1_w.T
    fc1bt = np.tile(fc1_b, 2)[:, None].astype(np.float32)

    g1 = np.exp(-1.0 / (gparam.astype(np.float64) ** 2 + 1e-8)).astype(np.float32)
    # band-block-diagonal (128 x 128) per layer
    wck = np.zeros((DEPTH, 128, 128), np.float32)
    wgk = np.zeros((DEPTH, 128, 128), np.float32)
    cbk = np.zeros((DEPTH, 128, 1), np.float32)
    for k in range(DEPTH):
        wck[k, 0:64, 0:64] = conv_w[k]
        wck[k, 64:128, 64:128] = conv_w[k]
        wgk[k] = wck[k] * g1[k]
        cbk[k, 0:64, 0] = conv_b[k]
        cbk[k, 64:128, 0] = conv_b[k]

    wfc2 = np.zeros((128, 64), np.float32)
    wfc2[0:64, 0:32] = fc2_w.T
    wfc2[64:128, 32:64] = fc2_w.T
    fc2bt = np.zeros((128, 1), np.float32)
    fc2bt[0:32, 0] = fc2_b
    fc2bt[32:64, 0] = fc2_b
    wfc3 = np.zeros((128, 32), np.float32)
    wfc3[0:32, 0:OUTC] = fc3_w.T
    wfc3[32:64, 16:16 + OUTC] = fc3_w.T
    fc3bt = np.zeros((128, 1), np.float32)
    fc3bt[0:OUTC, 0] = fc3_b
    fc3bt[16:16 + OUTC, 0] = fc3_b

    # LSTM(0-input) fixed point -> halo feats constant
    def sig(v):
        return 1.0 / (1.0 + np.exp(-v))
    h0 = c0 = h1 = c1 = np.zeros(HH, np.float32)
    for _ in range(T):
        z = h0 @ Whh0.T + bih0 + bhh0
        i_, f_, g_, o_ = np.split(z, 4)
        c0 = sig(f_) * c0 + sig(i_) * np.tanh(g_)
        h0 = sig(o_) * np.tanh(c0)
        z = h0 @ Wih1.T + bih1 + h1 @ Whh1.T + bhh1
        i_, f_, g_, o_ = np.split(z, 4)
        c1 = sig(f_) * c1 + sig(i_) * np.tanh(g_)
        h1 = sig(o_) * np.tanh(c1)
    c_feats = np.maximum(h1 @ fc1_w.T + fc1_b, 0.0).astype(np.float32)

    b1m = np.zeros((128, 512), np.float32)
    for q in range(4):
        for G in range(4):
            b1m[32 * q + 10, 128 * G + 32 * q:128 * G + 32 * q + 32] = \
                b1p[32 * G:32 * G + 32]

    kbf = np.zeros((128, 3744), np.float32)
    kbf[:, 0:512] = w0x
    kbf[:, 512:1024] = w0h
    kbf[:, 1024:1536] = w1i
    kbf[:, 1536:2048] = w1h
    kbf[:, 2048:2112] = wfc1
    kbf[:, 2112:2624] = wck.transpose(1, 0, 2).reshape(128, 512)
    kbf[:, 2624:3136] = wgk.transpose(1, 0, 2).reshape(128, 512)
    kbf[:, 3136:3200] = wfc2
    kbf[:, 3200:3232] = wfc3
    kbf[:, 3232:3744] = b1m
    kf32 = np.zeros((128, 12), np.float32)
    kf32[:, 0:4] = b1t
    kf32[:, 4:5] = fc1bt
    kf32[:, 5:9] = cbk.transpose(1, 0, 2).reshape(128, 4)
    kf32[:, 9:10] = fc2bt
    kf32[:, 10:11] = fc3bt
    shared = dict(kbf=kbf.astype(BF), kf32=kf32)
    return shared, c_feats, g1


def _arrange_x(xb, r0, R):
    """xb: (T, C, GRID, CW) one batch -> (R//8, T, 128, 512) quad-tile layout.
    Slab rows r0:r0+R (clamped, zero-padded); part 32q+10 = 1.0 (bias row)."""
    T_, C_, G_, CW_ = xb.shape
    slab = np.zeros((T_, C_, R, CW_), np.float32)
    lo, hi = max(r0, 0), min(r0 + R, G_)
    slab[:, :, lo - r0:hi - r0, :] = xb[:, :, lo:hi, :]
    out = np.zeros((R // 8, T_, 128, 512), np.float32)
    # (T, C, R, CW) -> quads j, chunks q (2 rows each)
    s = slab.reshape(T_, C_, R // 8, 4, 2 * CW_)
    out.reshape(R // 8, T_, 4, 32, 512)[:, :, :, 0:C_, :] = \
        s.transpose(2, 0, 3, 1, 4)
    out.reshape(R // 8, T_, 4, 32, 512)[:, :, :, C_, :] = 1.0
    return out.astype(BF)


def _host_prep(x, shared, c_feats, R=72):
    """Per-core input dicts: x slabs + halo-correction vectors."""
    own = R - 8
    B = x.shape[0]
    in_maps = []
    nblk = NCORES // B
    for core in range(NCORES):
        b, rb = core // nblk, core % nblk
        r0 = rb * own - 4
        xs = _arrange_x(x[b], r0, R)
        kf = shared["kf32"].copy()
        kf[:, 11] = 1.0
        if rb == 0:
            kf[0:64, 11] = 0.0
        if rb == nblk - 1:
            kf[64:128, 11] = 0.0
        m = dict(shared)
        m["x"] = xs
        m["kf32"] = kf
        in_maps.append(m)
    return in_maps


_CACHE = {}
TRACE = False
LAST_EXEC_NS = None
LAST_TRACE = None


def _run_in_subprocess(in_maps):
    """Run the SPMD program in a child process with a clean jax env.

    The grading/reference process often pins JAX_PLATFORMS=cpu, which breaks
    the axon PJRT compile hook; a child with a scrubbed env always sees the
    8 NeuronCores."""
    import pickle
    import subprocess
    import tempfile

    workdir = tempfile.mkdtemp(prefix="pfk_")
    inp = os.path.join(workdir, "in.pkl")
    outp = os.path.join(workdir, "out.pkl")
    with open(inp, "wb") as f:
        pickle.dump({"in_maps": in_maps, "gw1": GW1, "trace": TRACE}, f,
                    protocol=4)
    env = dict(os.environ)
    env.pop("JAX_PLATFORMS", None)
    env.pop("JAX_PLATFORM_NAME", None)
    subprocess.run([sys.executable, os.path.abspath(__file__),
                    "--worker", inp, outp], check=True, env=env)
    with open(outp, "rb") as f:
        return pickle.load(f)


def _worker(inp, outp):
    import pickle
    import time as _time
    import types

    # the trimmed axon container lacks antenv.axon_hooks (NTFF profiling);
    # stub it so trace=True degrades to no-trace instead of crashing.
    if "antenv.axon_hooks" not in sys.modules:
        stub = types.ModuleType("antenv.axon_hooks")
        stub.get_axon_ntff_profile_hook = lambda: None
        sys.modules["antenv.axon_hooks"] = stub

    with open(inp, "rb") as f:
        payload = pickle.load(f)
    global GW1, TRACE
    GW1 = payload["gw1"]
    TRACE = payload["trace"]
    from concourse.bass_utils import run_bass_kernel_spmd
    nc = build_nc(R=72)
    res = run_bass_kernel_spmd(nc, payload["in_maps"], list(range(NCORES)))
    times = []
    n_rep = int(os.environ.get("KREPS", "3"))
    for _ in range(n_rep):
        t0 = _time.perf_counter()
        res = run_bass_kernel_spmd(nc, payload["in_maps"], list(range(NCORES)))
        times.append(_time.perf_counter() - t0)
    out = {
        "outs": [np.asarray(res.results[i]["out"]) for i in range(NCORES)],
        "exec_time_ns": int(min(times) * 1e9) if times else None,
        "trace": None,
        "times": times,
    }
    with open(outp, "wb") as f:
        pickle.dump(out, f, protocol=4)


def kernel(x, edge_src, edge_tgt, edge_attr, Wih0, Whh0, bih0, bhh0,
           Wih1, Whh1, bih1, bhh1, fc1_w, fc1_b, conv_w, conv_b, gparam,
           fc2_w, fc2_b, fc3_w, fc3_b):
    x = np.ascontiguousarray(np.asarray(x, np.float32))
    B = x.shape[0]
    R, own = 72, 64
    shared, c_feats, g1 = _prep_shared(
        *[np.asarray(a, np.float32) for a in
          (Wih0, Whh0, bih0, bhh0, Wih1, Whh1, bih1, bhh1, fc1_w, fc1_b,
           conv_w, conv_b, gparam, fc2_w, fc2_b, fc3_w, fc3_b)])
    in_maps = _host_prep(x, shared, c_feats, R=R)

    global GW1
    GW1 = [float(v) for v in g1]

    global LAST_EXEC_NS, LAST_TRACE
    res = _run_in_subprocess(in_maps)
    LAST_EXEC_NS = res.get("exec_time_ns")
    LAST_TRACE = res.get("trace")
    nblk = NCORES // B
    full = np.zeros((B, OUTC, GRID, GRID), np.float32)
    for core in range(NCORES):
        b, rb = core // nblk, core % nblk
        o = res["outs"][core]  # (own//2, OUTC, 2, CW)
        full[b, :, rb * own:(rb + 1) * own, :] = \
            o.transpose(1, 0, 2, 3).reshape(OUTC, own, GRID)
    return full[:, None].astype(np.float32)


if __name__ == "__main__":
    if len(sys.argv) == 4 and sys.argv[1] == "--worker":
        _worker(sys.argv[2], sys.argv[3])


# revision 33
# speedup vs baseline: 1.4426x; 1.2170x over previous
"""PhaseFieldPredictor on 8 Trainium2 NeuronCores via Bass/Tile.

Sharding: core k -> (batch b=k//4, row-block rb=k%4). Each core computes a
72-row slab (64 owned rows + 4 halo rows each side, zero-padded off-grid) of
the 256x256 grid: per-node 2-layer LSTM (T=5), fc1, 4 gated-GNN layers, head.

Key structure on-device (per core):
 - LSTM is "gate-major": nodes processed in quads of 4 chunks x 512 nodes;
   chunk q lives on SBUF partitions 32q:32q+32, and each gate G of all 4
   chunks forms one 128-partition tile => full-width ACT/DVE ops.
   Matmuls are 32x32 diagonal tile_position=(32q,32q) ops; L0 bias rides an
   ones-row in the x tile (K=11), L1 biases ride the activation bias port.
 - GNN: the 8-neighbor gaussian-gated conv is exactly a separable
   [g1,1,g1] (x) [g1,1,g1] stencil (g1 = exp(-1/(g^2+1e-8)), diag weight
   g1^2) followed by a 64x64 channel matmul.  The W-direction pass is fused
   into the PE as 3 shifted accumulating matmuls (weights {g1*W, W, g1*W});
   the H-direction pass + bias + relu run on DVE as 3 fused ops.
   Node layout: channels on partitions, two row-bands stacked (band A rows
   0:40 on partitions 0:64, band B rows 32:72 on partitions 64:128), each row
   padded to 258 cols with zero guard columns.
"""
import os
import sys

import numpy as np

for _p in ("/opt/trn_rl_repo", "/root/.axon_site/_ro/trn_rl_repo"):
    if os.path.isdir(_p) and _p not in sys.path:
        sys.path.insert(0, _p)

import ml_dtypes

BF = ml_dtypes.bfloat16

# ---------------- configuration ----------------
GRID = 256
T, C, HH, WID, KW, OUTC = 5, 10, 32, 64, 32, 10
DEPTH = 4
NCORES = 8

# torch gate order in weight rows: i,f,g,o ; our slot order: i,f,o,g
_PERM = np.r_[0:32, 32:64, 96:128, 64:96]


def _geom(R):
    """Row geometry for a slab of R rows (R-8 owned)."""
    own = R - 8
    arows = own // 2 + 8          # band A rows 0:arows
    b0 = own // 2                 # band B rows b0:R
    brows = R - b0
    assert arows == brows
    return own, arows, b0


def _patch_drain(TileContext):
    """Walrus in this container rejects the Tile exit-drain's wide sem-wait
    list ('Too many sync wait commands').  Split the waits over chained
    sync-engine nops (<=4 waits each, strict FIFO on the sequencer), then
    emit a bare drain."""
    if getattr(TileContext, "_drain_patched", False):
        return
    from concourse.vector_clock import ScopedClock, VectorClock

    def _drain_and_barrier(self, tick_clock, wait_clock):
        gc = tick_clock.global_clock
        n = len(gc)
        for lo in range(0, n, 1):
            sub = [0] * n
            any_set = False
            for i in range(lo, min(lo + 1, n)):
                sub[i] = gc[i]
                any_set = any_set or gc[i] > 0
            if not any_set:
                continue
            nop = self.nc.sync.nop(nofuse=True)
            wait_clock.add_sem_waits(nop.ins,
                                     ScopedClock({None: VectorClock(sub)}))
        self.nc.sync.drain()
        self.nc.all_engine_barrier()
        assert self.sems is not None
        popped = self.nc._tile_sem_poison_stack.pop()
        assert popped is self._sem_poison
        self.nc.clear_and_free_semaphores(list(self.sems.allocated().values()))
        self.nc.all_engine_barrier()

    TileContext._drain_and_barrier = _drain_and_barrier
    TileContext._drain_patched = True


def _split_wide_waits(nc, mybir):
    """Walrus codegen in this container caps sem-waits per instruction (1 for
    DMA pseudo-instructions, small for others).  Move wide wait lists onto
    preceding same-engine nops (<=4 waits each; sequencers are in-order, so
    the fence is preserved)."""
    n_fix = 0
    for bb in nc.m.functions[0].blocks:
        insts = bb.instructions
        k = 0
        while k < len(insts):
            ins = insts[k]
            si = ins.sync_info
            if si is not None:
                waits = list(si.on_wait)
                if len(waits) > 1:
                    # keep the last wait on the instruction itself
                    for w in waits[:-1]:
                        nop = mybir.InstNoOp(
                            name=f"I-wfix-{n_fix}", ins=[], outs=[])
                        n_fix += 1
                        nop.engine = ins.engine
                        nop.sync_info = mybir.SyncInfo(
                            on_wait=[w], on_update=[])
                        insts.insert(k, nop)
                        k += 1
                    ins.sync_info = mybir.SyncInfo(
                        on_wait=[waits[-1]], on_update=list(si.on_update))
            k += 1
    return n_fix


def build_nc(R=72, CW=GRID, debug=False, fix_waits=True):
    import concourse.bass as bass
    import concourse.mybir as mybir

    from concourse.tile import TileContext
    _patch_drain(TileContext)

    F32 = mybir.dt.float32
    BF16 = mybir.dt.bfloat16
    AF = mybir.ActivationFunctionType
    ALU = mybir.AluOpType

    own, arows, b0 = _geom(R)
    CP = CW + 2                   # padded row pitch
    FD = arows * CP               # band free dim
    NQ = R // 8                   # quads
    nchunk2 = R // 2              # 2-row chunks in slab

    nc = bass.Bass()
    x_in = nc.declare_dram_parameter("x", [R // 8, T, 128, 512], BF16, isOutput=False)
    kbf = nc.declare_dram_parameter("kbf", [128, 3744], BF16, isOutput=False)
    kf32 = nc.declare_dram_parameter("kf32", [128, 12], F32, isOutput=False)
    out_d = nc.declare_dram_parameter("out", [own // 2, OUTC, 2, CW], F32,
                                      isOutput=True)

    with TileContext(nc) as tc:
        from contextlib import ExitStack
        with ExitStack() as es:
            cpool = es.enter_context(tc.tile_pool(name="const", bufs=1))
            bpool = es.enter_context(tc.tile_pool(name="band", bufs=1))
            wpool = es.enter_context(tc.tile_pool(name="work", bufs=5))
            xpool = es.enter_context(tc.tile_pool(name="xst", bufs=3))
            spool = es.enter_context(tc.tile_pool(name="state", bufs=3))
            gwpool = es.enter_context(tc.tile_pool(name="gwork", bufs=2))
            zps = ExitStack()
            zpool = zps.enter_context(tc.tile_pool(name="zps", bufs=1, space="PSUM"))

            # ---- constants (host-packed block-diagonal, two DMAs) ----
            # bf16 col map: w0x 0:512 | w0h 512:1024 | w1i 1024:1536 |
            #   w1h 1536:2048 | wfc1 2048:2112 | wc 2112:2624 | wg 2624:3136 |
            #   wfc2 3136:3200 | wfc3 3200:3232
            tKB = cpool.tile([128, 3744], BF16, tag="tKB")
            tKF = cpool.tile([128, 12], F32, tag="tKF")
            nc.sync.dma_start(out=tKB[:], in_=kbf[:])
            nc.sync.dma_start(out=tKF[:], in_=kf32[:])
            tW0x = tKB[:, 0:512]
            tW0h = tKB[:, 512:1024]
            tW1i = tKB[:, 1024:1536]
            tW1h = tKB[:, 1536:2048]
            tWfc1 = tKB[:, 2048:2112]
            tWc = tKB[:, 2112:2624]
            tWg = tKB[:, 2624:3136]
            tWfc2 = tKB[:, 3136:3200]
            tWfc3 = tKB[:, 3200:3232]
            tB1m = tKB[:, 3232:3744]
            # f32 col map: b1t 0:4 | fc1b 4 | cbk 5:9 | fc2b 9 | fc3b 10 |
            #   corr(mask) 11
            tB1 = tKF[:, 0:4]
            tFc1b = tKF[:, 4:5]
            tCb = tKF[:, 5:9]
            tFc2b = tKF[:, 9:10]
            tFc3b = tKF[:, 10:11]
            tCorr = tKF[:, 11:12]

            # ---- band tiles ----
            feats = bpool.tile([128, FD], BF16, tag="feats")
            mstage = bpool.tile([128, FD], BF16, tag="mstage")
            tmp = bpool.tile([128, FD - 2 * CP], BF16, tag="tmp")
            nc.vector.memset(feats[:], 0.0)
            nc.vector.memset(mstage[:], 0.0)
            nc.vector.memset(tmp[:], 0.0)

            # ---- LSTM + fc1, per quad of 8 rows ----
            for j in range(NQ):
                xts = []
                for t in range(T):
                    # host pre-arranged: parts 32q:32q+10 = x channels of chunk
                    # q (2 rows), part 32q+10 = ones row carrying the L0 bias
                    xt = xpool.tile([128, 512], BF16, tag=f"xt{t}")
                    nc.sync.dma_start(out=xt[:], in_=x_in[j, t])
                    xts.append(xt)

                # ----- pass A: layer 0, all T steps (h0 kept per step) ----
                h0a = spool.tile([128, 512 * T], BF16, tag="h0a")
                c0 = spool.tile([128, 512], BF16, tag="c0")
                for t in range(T):
                    z0 = zpool.tile([128, 2048], F32, tag="z0")
                    for G in (3, 0, 1, 2):
                        nc.tensor.matmul(
                            z0[:, 512 * G:512 * G + 512],
                            lhsT=tW0x[:, 128 * G:128 * G + 128],
                            rhs=xts[t][:],
                            start=True, stop=(t == 0))
                        if t > 0:
                            nc.tensor.matmul(
                                z0[:, 512 * G:512 * G + 512],
                                lhsT=tW0h[:, 128 * G:128 * G + 128],
                                rhs=h0a[:, 512 * (t - 1):512 * t],
                                start=False, stop=True)
                    sig0 = wpool.tile([128, 1536], BF16, tag="sig0")
                    tg0 = wpool.tile([128, 512], BF16, tag="tg0")
                    nc.scalar.activation(tg0[:], z0[:, 1536:2048], AF.Tanh)
                    nc.scalar.activation(sig0[:], z0[:, 0:1536], AF.Sigmoid)
                    if t == 0:
                        nc.vector.tensor_mul(c0[:], tg0[:], sig0[:, 0:512])
                    else:
                        pp = wpool.tile([128, 512], BF16, tag="pp")
                        qq = wpool.tile([128, 512], BF16, tag="qq")
                        nc.vector.tensor_mul(pp[:], tg0[:], sig0[:, 0:512])
                        nc.vector.tensor_mul(qq[:], c0[:], sig0[:, 512:1024])
                        nc.vector.tensor_add(c0[:], pp[:], qq[:])
                    tc0 = wpool.tile([128, 512], BF16, tag="tc0")
                    nc.scalar.activation(tc0[:], c0[:], AF.Tanh)
                    nc.vector.tensor_mul(h0a[:, 512 * t:512 * t + 512],
                                         sig0[:, 1024:1536], tc0[:])

                # ----- pass B: layer 1, all T steps ----
                h1 = spool.tile([128, 512], BF16, tag="h1")
                c1 = spool.tile([128, 512], BF16, tag="c1")
                for t in range(T):
                    z1 = zpool.tile([128, 2048], F32, tag="z1")
                    for G in (3, 0, 1, 2):
                        nc.tensor.matmul(
                            z1[:, 512 * G:512 * G + 512],
                            lhsT=tB1m[:, 128 * G:128 * G + 128],
                            rhs=xts[t][:],
                            start=True, stop=False)
                        nc.tensor.matmul(
                            z1[:, 512 * G:512 * G + 512],
                            lhsT=tW1i[:, 128 * G:128 * G + 128],
                            rhs=h0a[:, 512 * t:512 * t + 512],
                            start=False, stop=(t == 0))
                        if t > 0:
                            nc.tensor.matmul(
                                z1[:, 512 * G:512 * G + 512],
                                lhsT=tW1h[:, 128 * G:128 * G + 128],
                                rhs=h1[:],
                                start=False, stop=True)
                    sig1 = wpool.tile([128, 1536], BF16, tag="sig1")
                    tg1 = wpool.tile([128, 512], BF16, tag="tg1")
                    nc.scalar.activation(tg1[:], z1[:, 1536:2048], AF.Tanh)
                    nc.scalar.activation(sig1[:], z1[:, 0:1536], AF.Sigmoid)
                    if t == 0:
                        nc.vector.tensor_mul(c1[:], tg1[:], sig1[:, 0:512])
                    else:
                        pp1 = wpool.tile([128, 512], BF16, tag="pp1")
                        qq1 = wpool.tile([128, 512], BF16, tag="qq1")
                        nc.vector.tensor_mul(pp1[:], tg1[:], sig1[:, 0:512])
                        nc.vector.tensor_mul(qq1[:], c1[:], sig1[:, 512:1024])
                        nc.vector.tensor_add(c1[:], pp1[:], qq1[:])
                    tc1 = wpool.tile([128, 512], BF16, tag="tc1")
                    nc.scalar.activation(tc1[:], c1[:], AF.Tanh)
                    nc.vector.tensor_mul(h1[:], sig1[:, 1024:1536], tc1[:])

                # ----- fc1 for this quad's 4 chunks -----
                fcp = zpool.tile([128, 2048], F32, tag="z1")
                for q in range(4):
                    m = 4 * j + q          # 2-row chunk index; rows 2m:2m+2
                    p = 32 * q
                    for band in (0, 1):
                        if band == 0 and 2 * m + 2 > arows:
                            continue
                        if band == 1 and 2 * m < b0:
                            continue
                        cs = 64 * band
                        nc.tensor.matmul(
                            fcp[cs:cs + 64, 512 * q:512 * q + 512],
                            lhsT=tWfc1[p:p + 32, :],
                            rhs=h1[p:p + 32, :],
                            start=True, stop=True, tile_position=(p, cs))
                        row = 2 * m - band * b0
                        dst = feats[cs:cs + 64, row * CP:(row + 2) * CP] \
                            .rearrange("p (r w) -> p r w", r=2)[:, :, 1:1 + CW]
                        nc.vector.tensor_scalar(
                            dst, fcp[cs:cs + 64, 512 * q:512 * q + 512]
                            .rearrange("p (r w) -> p r w", r=2),
                            tFc1b[cs:cs + 64, 0:1], 0.0, ALU.add, ALU.max)

            # ---- halo mask: zero off-grid rows (per-core 0/1 vector) ----
            def mask_halo():
                for band, r_lo in ((0, 0), (1, arows - 4)):
                    cs = 64 * band
                    sl = feats[cs:cs + 64, r_lo * CP:(r_lo + 4) * CP]
                    nc.vector.tensor_scalar(sl, sl, tCorr[cs:cs + 64, 0:1],
                                            None, ALU.mult)
            mask_halo()
            if debug:
                dbg = nc.declare_dram_parameter(
                    "dbg", [DEPTH + 1, 128, FD], F32, isOutput=True)
                nc.gpsimd.dma_start(out=dbg[0], in_=feats[:])

            zps.close()  # free LSTM psum banks
            gpool = es.enter_context(tc.tile_pool(name="gps", bufs=2, space="PSUM"))
            hpool = es.enter_context(tc.tile_pool(name="hps", bufs=2, space="PSUM"))

            # ---- GNN layers ----
            nck = (FD + 511) // 512
            for k in range(DEPTH):
                if k > 0:
                    mask_halo()
                wc = tWc[:, 128 * k:128 * k + 128]
                wg = tWg[:, 128 * k:128 * k + 128]
                for ci in range(nck):
                    lo = 512 * ci
                    ln = min(512, FD - lo)
                    mp = gpool.tile([128, 512], F32, tag="mp")
                    llo = max(lo - 1, 0)
                    rn = ln if lo + ln < FD else ln - 1
                    nc.tensor.matmul(
                        mp[:, 0:ln], lhsT=wc,
                        rhs=feats[:, lo:lo + ln],
                        start=True, stop=False)
                    nc.tensor.matmul(
                        mp[:, llo - lo + 1:ln], lhsT=wg,
                        rhs=feats[:, llo:lo + ln - 1],
                        start=False, stop=False)
                    nc.tensor.matmul(
                        mp[:, 0:rn], lhsT=wg,
                        rhs=feats[:, lo + 1:lo + 1 + rn],
                        start=False, stop=True)
                    nc.scalar.activation(mstage[:, lo:lo + ln],
                                         mp[:, 0:ln], AF.Copy)
                # H-pass: rows 1..arows-1 ; out = relu(m + g1*(up+dn) + b)
                # done in column blocks so the next layer's W-pass chunks can
                # start before the whole band is swept
                nfd = FD - 2 * CP
                nblk = 4
                bsz = (nfd + nblk - 1) // nblk
                for bi in range(nblk):
                    blo = bi * bsz
                    bn = min(bsz, nfd - blo)
                    tb = tmp[:, blo:blo + bn]
                    nc.vector.tensor_add(tb, mstage[:, blo:blo + bn],
                                         mstage[:, blo + 2 * CP:blo + 2 * CP + bn])
                    nc.vector.scalar_tensor_tensor(
                        tb, tb, GW1[k], mstage[:, blo + CP:blo + CP + bn],
                        ALU.mult, ALU.add)
                    if k != DEPTH - 1:
                        nc.vector.tensor_scalar(
                            feats[:, blo + CP:blo + CP + bn], tb,
                            tCb[:, k:k + 1], 0.0, ALU.add, ALU.max)
                    else:
                        nc.vector.tensor_scalar(
                            feats[:, blo + CP:blo + CP + bn], tb,
                            tCb[:, k:k + 1], None, ALU.add)
                # re-zero guard cols (both bands, all rows)
                nc.vector.memset(
                    feats.rearrange("p (r w) -> p r w", r=arows)[:, :, 0:1], 0.0)
                nc.vector.memset(
                    feats.rearrange("p (r w) -> p r w", r=arows)[:, :, CP - 1:CP],
                    0.0)
                if debug:
                    nc.gpsimd.dma_start(out=dbg[k + 1], in_=feats[:])

            # ---- head: owned rows = band rows 4 : 4+own/2 on each band ----
            for m in range(own // 4):
                lo = (4 + 2 * m) * CP
                hp = hpool.tile([64, 512], F32, tag="hp")
                rhs_ap = feats[:, lo:lo + 2 * CP] \
                    .rearrange("p (r w) -> p r w", r=2)[:, :, 1:1 + CW]
                nc.tensor.matmul(hp[:], lhsT=tWfc2, rhs=rhs_ap,
                                 start=True, stop=True)
                r2 = gwpool.tile([64, 512], BF16, tag="r2")
                nc.scalar.activation(r2[:], hp[:], AF.Relu,
                                     bias=tFc2b[0:64, 0:1])
                op3 = hpool.tile([32, 512], F32, tag="op3")
                nc.tensor.matmul(op3[:], lhsT=tWfc3[0:64, :], rhs=r2[:],
                                 start=True, stop=True)
                ot = gwpool.tile([32, 512], F32, tag="ot")
                nc.scalar.activation(ot[:], op3[:], AF.Identity,
                                     bias=tFc3b[0:32, 0:1])
                # out block: band A (parts 0:10) -> m ; band B (16:26) -> +own/4
                for band in (0, 1):
                    cs = 16 * band
                    osrc = ot[cs:cs + OUTC, :].rearrange(
                        "p (r w) -> p r w", r=2)
                    nc.sync.dma_start(
                        out=out_d[band * (own // 4) + m], in_=osrc)

    if fix_waits:
        _split_wide_waits(nc, mybir)
    return nc


GW1 = [1.0] * DEPTH  # per-layer g1 scalars, set by host before build


def _prep_shared(Wih0, Whh0, bih0, bhh0, Wih1, Whh1, bih1, bhh1,
                 fc1_w, fc1_b, conv_w, conv_b, gparam, fc2_w, fc2_b,
                 fc3_w, fc3_b):
    """Weight/bias tiles shared by all cores; returns (dict, c_feats, g1)."""
    Wih0p, Whh0p = Wih0[_PERM], Whh0[_PERM]
    Wih1p, Whh1p = Wih1[_PERM], Whh1[_PERM]
    b0p = (bih0 + bhh0)[_PERM]
    b1p = (bih1 + bhh1)[_PERM]

    # block-diagonal over the 4 chunk-slots: gate-G matmul lhsT is
    # (128 K x 128 M) with chunk q's block at [32q:32q+*, 128G+32q:+32]
    w0x = np.zeros((128, 512), np.float32)
    w0h = np.zeros((128, 512), np.float32)
    w1i = np.zeros((128, 512), np.float32)
    w1h = np.zeros((128, 512), np.float32)
    b1t = np.zeros((128, 4), np.float32)
    for q in range(4):
        for G in range(4):
            c0 = 128 * G + 32 * q
            w0x[32 * q:32 * q + 10, c0:c0 + 32] = Wih0p[32 * G:32 * G + 32].T
            w0x[32 * q + 10, c0:c0 + 32] = b0p[32 * G:32 * G + 32]
            w0h[32 * q:32 * q + 32, c0:c0 + 32] = Whh0p[32 * G:32 * G + 32].T
            w1i[32 * q:32 * q + 32, c0:c0 + 32] = Wih1p[32 * G:32 * G + 32].T
            w1h[32 * q:32 * q + 32, c0:c0 + 32] = Whh1p[32 * G:32 * G + 32].T
            b1t[32 * q:32 * q + 32, G] = b1p[32 * G:32 * G + 32]

    wfc1 = np.zeros((128, 64), np.float32)
    for q in range(4):
        wfc1[32 * q:32 * q + 32] = fc1_w.T
    fc1bt = np.tile(fc1_b, 2)[:, None].astype(np.float32)

    g1 = np.exp(-1.0 / (gparam.astype(np.float64) ** 2 + 1e-8)).astype(np.float32)
    # band-block-diagonal (128 x 128) per layer
    wck = np.zeros((DEPTH, 128, 128), np.float32)
    wgk = np.zeros((DEPTH, 128, 128), np.float32)
    cbk = np.zeros((DEPTH, 128, 1), np.float32)
    for k in range(DEPTH):
        wck[k, 0:64, 0:64] = conv_w[k]
        wck[k, 64:128, 64:128] = conv_w[k]
        wgk[k] = wck[k] * g1[k]
        cbk[k, 0:64, 0] = conv_b[k]
        cbk[k, 64:128, 0] = conv_b[k]

    wfc2 = np.zeros((128, 64), np.float32)
    wfc2[0:64, 0:32] = fc2_w.T
    wfc2[64:128, 32:64] = fc2_w.T
    fc2bt = np.zeros((128, 1), np.float32)
    fc2bt[0:32, 0] = fc2_b
    fc2bt[32:64, 0] = fc2_b
    wfc3 = np.zeros((128, 32), np.float32)
    wfc3[0:32, 0:OUTC] = fc3_w.T
    wfc3[32:64, 16:16 + OUTC] = fc3_w.T
    fc3bt = np.zeros((128, 1), np.float32)
    fc3bt[0:OUTC, 0] = fc3_b
    fc3bt[16:16 + OUTC, 0] = fc3_b

    # LSTM(0-input) fixed point -> halo feats constant
    def sig(v):
        return 1.0 / (1.0 + np.exp(-v))
    h0 = c0 = h1 = c1 = np.zeros(HH, np.float32)
    for _ in range(T):
        z = h0 @ Whh0.T + bih0 + bhh0
        i_, f_, g_, o_ = np.split(z, 4)
        c0 = sig(f_) * c0 + sig(i_) * np.tanh(g_)
        h0 = sig(o_) * np.tanh(c0)
        z = h0 @ Wih1.T + bih1 + h1 @ Whh1.T + bhh1
        i_, f_, g_, o_ = np.split(z, 4)
        c1 = sig(f_) * c1 + sig(i_) * np.tanh(g_)
        h1 = sig(o_) * np.tanh(c1)
    c_feats = np.maximum(h1 @ fc1_w.T + fc1_b, 0.0).astype(np.float32)

    b1m = np.zeros((128, 512), np.float32)
    for q in range(4):
        for G in range(4):
            b1m[32 * q + 10, 128 * G + 32 * q:128 * G + 32 * q + 32] = \
                b1p[32 * G:32 * G + 32]

    kbf = np.zeros((128, 3744), np.float32)
    kbf[:, 0:512] = w0x
    kbf[:, 512:1024] = w0h
    kbf[:, 1024:1536] = w1i
    kbf[:, 1536:2048] = w1h
    kbf[:, 2048:2112] = wfc1
    kbf[:, 2112:2624] = wck.transpose(1, 0, 2).reshape(128, 512)
    kbf[:, 2624:3136] = wgk.transpose(1, 0, 2).reshape(128, 512)
    kbf[:, 3136:3200] = wfc2
    kbf[:, 3200:3232] = wfc3
    kbf[:, 3232:3744] = b1m
    kf32 = np.zeros((128, 12), np.float32)
    kf32[:, 0:4] = b1t
    kf32[:, 4:5] = fc1bt
    kf32[:, 5:9] = cbk.transpose(1, 0, 2).reshape(128, 4)
    kf32[:, 9:10] = fc2bt
    kf32[:, 10:11] = fc3bt
    shared = dict(kbf=kbf.astype(BF), kf32=kf32)
    return shared, c_feats, g1


def _arrange_x(xb, r0, R):
    """xb: (T, C, GRID, CW) one batch -> (R//8, T, 128, 512) quad-tile layout.
    Slab rows r0:r0+R (clamped, zero-padded); part 32q+10 = 1.0 (bias row)."""
    T_, C_, G_, CW_ = xb.shape
    slab = np.zeros((T_, C_, R, CW_), np.float32)
    lo, hi = max(r0, 0), min(r0 + R, G_)
    slab[:, :, lo - r0:hi - r0, :] = xb[:, :, lo:hi, :]
    out = np.zeros((R // 8, T_, 128, 512), np.float32)
    # (T, C, R, CW) -> quads j, chunks q (2 rows each)
    s = slab.reshape(T_, C_, R // 8, 4, 2 * CW_)
    out.reshape(R // 8, T_, 4, 32, 512)[:, :, :, 0:C_, :] = \
        s.transpose(2, 0, 3, 1, 4)
    out.reshape(R // 8, T_, 4, 32, 512)[:, :, :, C_, :] = 1.0
    return out.astype(BF)


def _host_prep(x, shared, c_feats, R=72):
    """Per-core input dicts: x slabs + halo-correction vectors."""
    own = R - 8
    B = x.shape[0]
    in_maps = []
    nblk = NCORES // B
    for core in range(NCORES):
        b, rb = core // nblk, core % nblk
        r0 = rb * own - 4
        xs = _arrange_x(x[b], r0, R)
        kf = shared["kf32"].copy()
        kf[:, 11] = 1.0
        if rb == 0:
            kf[0:64, 11] = 0.0
        if rb == nblk - 1:
            kf[64:128, 11] = 0.0
        m = dict(shared)
        m["x"] = xs
        m["kf32"] = kf
        in_maps.append(m)
    return in_maps


_CACHE = {}
TRACE = False
LAST_EXEC_NS = None
LAST_TRACE = None


def _run_in_subprocess(in_maps):
    """Run the SPMD program in a child process with a clean jax env.

    The grading/reference process often pins JAX_PLATFORMS=cpu, which breaks
    the axon PJRT compile hook; a child with a scrubbed env always sees the
    8 NeuronCores."""
    import pickle
    import subprocess
    import tempfile

    workdir = tempfile.mkdtemp(prefix="pfk_")
    inp = os.path.join(workdir, "in.pkl")
    outp = os.path.join(workdir, "out.pkl")
    with open(inp, "wb") as f:
        pickle.dump({"in_maps": in_maps, "gw1": GW1, "trace": TRACE}, f,
                    protocol=4)
    env = dict(os.environ)
    env.pop("JAX_PLATFORMS", None)
    env.pop("JAX_PLATFORM_NAME", None)
    subprocess.run([sys.executable, os.path.abspath(__file__),
                    "--worker", inp, outp], check=True, env=env)
    with open(outp, "rb") as f:
        return pickle.load(f)


def _worker(inp, outp):
    import pickle
    import time as _time
    import types

    # the trimmed axon container lacks antenv.axon_hooks (NTFF profiling);
    # stub it so trace=True degrades to no-trace instead of crashing.
    if "antenv.axon_hooks" not in sys.modules:
        stub = types.ModuleType("antenv.axon_hooks")
        stub.get_axon_ntff_profile_hook = lambda: None
        sys.modules["antenv.axon_hooks"] = stub

    with open(inp, "rb") as f:
        payload = pickle.load(f)
    global GW1, TRACE
    GW1 = payload["gw1"]
    TRACE = payload["trace"]
    from concourse.bass_utils import run_bass_kernel_spmd
    nc = build_nc(R=72)
    res = run_bass_kernel_spmd(nc, payload["in_maps"], list(range(NCORES)))
    times = []
    n_rep = int(os.environ.get("KREPS", "1"))
    for _ in range(n_rep):
        t0 = _time.perf_counter()
        res = run_bass_kernel_spmd(nc, payload["in_maps"], list(range(NCORES)))
        times.append(_time.perf_counter() - t0)
    out = {
        "outs": [np.asarray(res.results[i]["out"]) for i in range(NCORES)],
        "exec_time_ns": int(min(times) * 1e9) if times else None,
        "trace": None,
        "times": times,
    }
    with open(outp, "wb") as f:
        pickle.dump(out, f, protocol=4)


def kernel(x, edge_src, edge_tgt, edge_attr, Wih0, Whh0, bih0, bhh0,
           Wih1, Whh1, bih1, bhh1, fc1_w, fc1_b, conv_w, conv_b, gparam,
           fc2_w, fc2_b, fc3_w, fc3_b):
    x = np.ascontiguousarray(np.asarray(x, np.float32))
    B = x.shape[0]
    R, own = 72, 64
    shared, c_feats, g1 = _prep_shared(
        *[np.asarray(a, np.float32) for a in
          (Wih0, Whh0, bih0, bhh0, Wih1, Whh1, bih1, bhh1, fc1_w, fc1_b,
           conv_w, conv_b, gparam, fc2_w, fc2_b, fc3_w, fc3_b)])
    in_maps = _host_prep(x, shared, c_feats, R=R)

    global GW1
    GW1 = [float(v) for v in g1]

    global LAST_EXEC_NS, LAST_TRACE
    res = _run_in_subprocess(in_maps)
    LAST_EXEC_NS = res.get("exec_time_ns")
    LAST_TRACE = res.get("trace")
    nblk = NCORES // B
    full = np.zeros((B, OUTC, GRID, GRID), np.float32)
    for core in range(NCORES):
        b, rb = core // nblk, core % nblk
        o = res["outs"][core]  # (own//2, OUTC, 2, CW)
        full[b, :, rb * own:(rb + 1) * own, :] = \
            o.transpose(1, 0, 2, 3).reshape(OUTC, own, GRID)
    return full[:, None].astype(np.float32)


if __name__ == "__main__":
    if len(sys.argv) == 4 and sys.argv[1] == "--worker":
        _worker(sys.argv[2], sys.argv[3])
